# revision 2
# baseline (speedup 1.0000x reference)
"""BitLinear (ternary-weight linear) Trainium2 kernel, v3.

Computes  Y = x @ ternarize(W).T + bias  (see reference.py).

Column-parallel over 8 cores (o_shard = 2048 per core). vs v2:
  - Hybrid-precision matmul: the first n8 k-tiles run as fp8e4 DoubleRow
    matmuls (x fp8 stationary pairs x wq fp8 moving pairs, 2 MACs/cell/
    cycle); the remaining k-tiles stay bf16-x-stationary x fp8-wq-moving.
    Ternary wq is exact in fp8; only the fp8 x rounding costs accuracy
    (host-verified: n8=16 -> rel 0.0137, n8=24 -> 0.0161 vs 2e-2 gate).
  - Phase A (global mean|W|) reads a bf16 copy of W (half the serial HBM
    time); host-verified the bf16-mean threshold flips 0 ternary values.
  - Phase B ternarizes from the f32 W stream and overlaps phase C: the
    matmul consumer chases wq production per k-tile.
"""

import numpy as np

import concourse.bass as bass
import concourse.bacc as bacc
import concourse.tile as tile
import concourse.mybir as mybir
import concourse.bass_isa as bass_isa
from concourse import bass_utils

F32 = mybir.dt.float32
BF16 = mybir.dt.bfloat16
FP8 = mybir.dt.float8e4
NP_BF16 = mybir.dt.np(mybir.dt.bfloat16)
NP_FP8 = mybir.dt.np(mybir.dt.float8e4)

N_CORES = 8
TOKENS = 8192
K_FEAT = 4096
OUT_FEAT = 16384

P = 128
NB = 512
N8 = 16  # k-tiles (of 32) computed in fp8 DoubleRow; rest bf16

THRESHOLD = 0.05
EPS = 1e-6

Sign = bass_isa.ActivationFunctionType.Sign if hasattr(bass_isa, "ActivationFunctionType") else None
if Sign is None:
    import bass_rust
    Sign = bass_rust.ActivationFunctionType.Sign

DR = mybir.MatmulPerfMode.DoubleRow


def _ldw_sig(inst):
    a = inst.ins[0]
    return (a.memref, a.offset, str(a.ap), str(a.dtype),
            str(inst.perf_mode), str(inst.is_transpose), str(inst.tile_position))


def _dedupe_ldweights(nc):
    """Remove PE LDWEIGHTS that reload the stationary operand already in the
    array (identical AP, only MATMULs in between)."""
    n_removed = 0
    for bb in nc.main_func.blocks:
        insts = bb.instructions
        last_sig = None
        pending_waits = []
        keep = []
        for inst in insts:
            if inst.engine != mybir.EngineType.PE:
                keep.append(inst)
                continue
            if isinstance(inst, mybir.InstLdweights):
                si = inst.sync_info
                has_updates = si is not None and len(si.on_update) > 0
                sig = _ldw_sig(inst)
                if sig == last_sig and not has_updates and not inst.ins[0].regs_read():
                    if si is not None and len(si.on_wait) > 0:
                        pending_waits.extend(si.on_wait)
                    n_removed += 1
                    continue
                last_sig = sig
            elif isinstance(inst, mybir.InstMatmult):
                pass
            else:
                last_sig = None
            if pending_waits:
                si = inst.sync_info
                if si is None:
                    inst.sync_info = mybir.SyncInfo(
                        on_wait=list(pending_waits), on_update=[]
                    )
                else:
                    si.on_wait = list(pending_waits) + list(si.on_wait)
                pending_waits = []
            keep.append(inst)
        assert not pending_waits, "trailing LDW waits with no PE successor"
        if len(keep) != len(insts):
            while len(insts):
                insts.pop()
            for inst in keep:
                insts.append(inst)
    return n_removed


# per-ktile ternarize engine assignment (pattern of 8):
#  D = DVE 2-pass (tensor_scalar ptr variants are DVE-only on trn2)
#  A = ACT 2xSign + DVE combine (tt add + ts*0.5)
#  B = ACT 2xSign + GpSimd combine (immediate-scalar ops only on Pool)
ENGINE_PATTERN = "AADAADAD"


def build_kernel(tokens=TOKENS, k_feat=K_FEAT, out_feat=OUT_FEAT, n_cores=N_CORES,
                 use_collective=True, compile=True, nb=NB, cache_salt=0,
                 dedupe_ldw=True, xbufs=2, wchunk_a=4, wchunk_b=2, wbufs=2,
                 obufs=2, y_dtype=BF16, engine_pattern=ENGINE_PATTERN, n8=N8):
    """Build + compile the per-core Bass program (SPMD, symmetric)."""
    o_shard = out_feat // n_cores
    t_tiles = tokens // P
    k_tiles = k_feat // P
    ob_tiles = o_shard // nb
    assert n8 % 2 == 0 and 0 <= n8 <= k_tiles
    nbf = k_tiles - n8

    nc = bacc.Bacc("TRN2", target_bir_lowering=False, debug=False, num_devices=n_cores)

    # xt8[tb, p, c, t] = fp8(x[tb*128 + t, c*128 + p]), c in [0, n8)
    xt8_d = nc.dram_tensor("xt8", [t_tiles, P, n8, P], FP8, kind="ExternalInput") if n8 else None
    # xtb[tb, p, c, t] = bf16(x[tb*128 + t, (n8+c)*128 + p]), c in [0, nbf)
    xtb_d = nc.dram_tensor("xtb", [t_tiles, P, nbf, P], BF16, kind="ExternalInput") if nbf else None
    # wtb[p, kt, o] = bf16(W[o_global, kt*128 + p])  (phase-A mean only)
    wtb_d = nc.dram_tensor("wtb", [P, k_tiles, o_shard], BF16, kind="ExternalInput")
    # wt[p, kt, o] = W[o_global, kt*128 + p]  (f32, phase-B ternarize)
    wt_d = nc.dram_tensor("wt", [P, k_tiles, o_shard], F32, kind="ExternalInput")
    bias_d = nc.dram_tensor("bias", [1, o_shard], F32, kind="ExternalInput")
    y_d = nc.dram_tensor("y", [tokens, o_shard], y_dtype, kind="ExternalOutput")

    with tile.TileContext(nc) as tc:
        with (
            tc.tile_pool(name="singles", bufs=1) as singles,
            tc.tile_pool(name="wq", bufs=1) as wq_pool,
            tc.tile_pool(name="wstage_a", bufs=wbufs) as wstage_a,
            tc.tile_pool(name="wstage_b", bufs=wbufs) as wstage_b,
            tc.tile_pool(name="b01", bufs=2) as b01_pool,
            tc.tile_pool(name="actp", bufs=2) as act_pool,
            tc.tile_pool(name="xp8", bufs=xbufs) as xpool8,
            tc.tile_pool(name="xpb", bufs=xbufs) as xpoolb,
            tc.tile_pool(name="op", bufs=obufs) as opool,
            tc.tile_pool(name="psum", bufs=2, space="PSUM") as psum_pool,
            tc.tile_pool(name="dram", bufs=1, space="DRAM") as dram,
        ):
            # ---------- Phase A: global scale = mean(|W|) from bf16 copy ----------
            n_chunks_a = k_tiles // wchunk_a
            acc = singles.tile([P, k_tiles], F32)
            for ci in range(n_chunks_a):
                w_i = wstage_a.tile([P, wchunk_a, o_shard], BF16, name="wsa")
                nc.sync.dma_start(w_i[:], wtb_d[:, ci * wchunk_a:(ci + 1) * wchunk_a, :])
                for k in range(wchunk_a):
                    kt = ci * wchunk_a + k
                    nc.vector.tensor_reduce(
                        acc[:, kt:kt + 1], w_i[:, k, :],
                        axis=mybir.AxisListType.X, op=mybir.AluOpType.add,
                        apply_absolute_value=True,
                    )
            colsum = singles.tile([P, 1], F32)
            nc.vector.tensor_reduce(
                colsum[:], acc[:], axis=mybir.AxisListType.X, op=mybir.AluOpType.add
            )
            # partition sum via PE: [1,1] = colsum.T @ ones
            ones = singles.tile([P, 1], F32)
            nc.vector.memset(ones[:], 1.0)
            ps_sc = psum_pool.tile([P, o_shard], F32, name="ps")
            nc.tensor.matmul(ps_sc[0:1, 0:1], colsum[:], ones[:])
            ssum8 = singles.tile([1, 8], F32)
            nc.vector.memset(ssum8[:], 0.0)
            for _ in range(cache_salt):  # perturb BIR hash for A/B compiles
                nc.vector.memset(ssum8[:, 7:8], 0.0)
            nc.vector.tensor_copy(ssum8[:, 0:1], ps_sc[0:1, 0:1])
            in_b = dram.tile([1, 8], F32)
            out_b = dram.tile([1, 8], F32)
            nc.gpsimd.dma_start(in_b[:], ssum8[:])
            if use_collective:
                nc.gpsimd.collective_compute(
                    "AllReduce",
                    mybir.AluOpType.add,
                    replica_groups=[list(range(n_cores))],
                    ins=[in_b.opt()],
                    outs=[out_b.opt()],
                )
            else:  # single-core / sim variant
                nc.gpsimd.dma_start(out_b[:], in_b[:])
            gsum = singles.tile([1, 8], F32)
            nc.gpsimd.dma_start(gsum[:], out_b[:])

            # thr = 0.05 * max(gsum/(out*k), eps); also need -thr
            scale_p0 = singles.tile([1, 1], F32)
            nc.vector.tensor_scalar(
                scale_p0[:], gsum[0:1, 0:1],
                1.0 / (out_feat * k_feat), EPS,
                op0=mybir.AluOpType.mult, op1=mybir.AluOpType.max,
            )
            thr_p0 = singles.tile([1, 1], F32)
            nthr_p0 = singles.tile([1, 1], F32)
            nc.vector.tensor_scalar_mul(thr_p0[:], scale_p0[:], THRESHOLD)
            nc.vector.tensor_scalar_mul(nthr_p0[:], scale_p0[:], -THRESHOLD)
            thr = singles.tile([P, 1], F32)
            nthr = singles.tile([P, 1], F32)
            nc.gpsimd.partition_broadcast(thr[:], thr_p0[:])
            nc.gpsimd.partition_broadcast(nthr[:], nthr_p0[:])

            # bias broadcast to all partitions
            bias_row = singles.tile([1, o_shard], F32)
            nc.sync.dma_start(bias_row[:], bias_d[:])
            bias_bc = singles.tile([P, o_shard], F32)
            nc.gpsimd.partition_broadcast(bias_bc[:], bias_row[:])

            # ---------- Phase B: ternarize f32 shard -> resident wq ----------
            # wq8[i]: [P, 2, o_shard] fp8 pair tiles for DoubleRow (k-tiles 2i, 2i+1)
            # wqb[c]: [P, o_shard] fp8 moving tiles for the bf16 part
            wq8 = [wq_pool.tile([P, 2, o_shard], FP8, name=f"wq8_{i}")
                   for i in range(n8 // 2)]
            wqb = [wq_pool.tile([P, o_shard], FP8, name=f"wqb_{c}")
                   for c in range(nbf)]

            def tern(dst, w_k, kt):
                eng = engine_pattern[kt % len(engine_pattern)]
                if eng in ("A", "B"):
                    s1 = act_pool.tile([P, o_shard], BF16, name="s1")
                    s2 = act_pool.tile([P, o_shard], BF16, name="s2")
                    # sign(w + t), sign(w - t); avg = ternary
                    nc.scalar.activation(s1[:], w_k, Sign, bias=thr[:])
                    nc.scalar.activation(s2[:], w_k, Sign, bias=nthr[:])
                    t2 = act_pool.tile([P, o_shard], BF16, name="t2")
                    e = nc.vector if eng == "A" else nc.gpsimd
                    e.tensor_tensor(t2[:], s1[:], s2[:], op=mybir.AluOpType.add)
                    e.tensor_scalar_mul(dst, t2[:], 0.5)
                else:
                    b01 = b01_pool.tile([P, o_shard], BF16, name="b01")
                    nq = o_shard // NB
                    for q in range(nq):
                        sl = slice(q * NB, (q + 1) * NB)
                        nc.vector.tensor_scalar(
                            b01[:, sl], w_k[:, sl], nthr[:], None,
                            op0=mybir.AluOpType.is_lt,
                        )
                        nc.vector.scalar_tensor_tensor(
                            dst[:, sl], w_k[:, sl], thr[:], b01[:, sl],
                            op0=mybir.AluOpType.is_gt,
                            op1=mybir.AluOpType.subtract,
                        )

            n_chunks_b = k_tiles // wchunk_b
            for ci in range(n_chunks_b):
                w_i = wstage_b.tile([P, wchunk_b, o_shard], F32, name="wsb")
                nc.sync.dma_start(w_i[:], wt_d[:, ci * wchunk_b:(ci + 1) * wchunk_b, :])
                for k in range(wchunk_b):
                    kt = ci * wchunk_b + k
                    if kt < n8:
                        dst = wq8[kt // 2][:, kt % 2, :]
                    else:
                        dst = wqb[kt - n8][:]
                    tern(dst, w_i[:, k, :], kt)

            # ---------- Phase C: hybrid matmul + bias ----------
            for tb in range(t_tiles):
                x8t = None
                xbt = None
                if n8:
                    x8t = xpool8.tile([P, n8, P], FP8, name="x8")
                    nc.sync.dma_start(x8t[:], xt8_d[tb])
                if nbf:
                    xbt = xpoolb.tile([P, nbf, P], BF16, name="xb")
                    nc.sync.dma_start(xbt[:], xtb_d[tb])
                ps = psum_pool.tile([P, o_shard], F32, name="ps")
                # fp8 DoubleRow part: stationary x pair, moving wq pair
                for i in range(n8 // 2):
                    lhsT = x8t[:, 2 * i:2 * i + 2, :]
                    for ob in range(ob_tiles):
                        nc.tensor.matmul(
                            ps[:, ob * nb:(ob + 1) * nb],
                            lhsT,
                            wq8[i][:, :, ob * nb:(ob + 1) * nb],
                            start=(i == 0),
                            stop=(nbf == 0 and i == n8 // 2 - 1),
                            perf_mode=DR,
                        )
                # bf16 part: stationary x tile, moving fp8 wq
                for c in range(nbf):
                    lhsT = xbt[:, c, :]
                    for ob in range(ob_tiles):
                        nc.tensor.matmul(
                            ps[:, ob * nb:(ob + 1) * nb],
                            lhsT,
                            wqb[c][:, ob * nb:(ob + 1) * nb],
                            start=(n8 == 0 and c == 0),
                            stop=(c == nbf - 1),
                        )
                ot = opool.tile([P, o_shard], y_dtype, name="ot")
                nc.vector.tensor_tensor(
                    ot[:], ps[:], bias_bc[:], op=mybir.AluOpType.add
                )
                nc.sync.dma_start(y_d[tb * P:(tb + 1) * P, :], ot[:])

    if dedupe_ldw:
        n = _dedupe_ldweights(nc)
        import logging
        logging.getLogger(__name__).info("dedupe_ldweights removed %d", n)
    if compile:
        nc.compile()
    return nc


def make_in_maps(x, weight, bias, tokens=TOKENS, k_feat=K_FEAT, out_feat=OUT_FEAT,
                 n_cores=N_CORES, n8=N8):
    """Host-side marshalling: shard + relayout + dtype-cast the full inputs."""
    o_shard = out_feat // n_cores
    t_tiles = tokens // P
    k_tiles = k_feat // P
    nbf = k_tiles - n8
    # x[t, k] -> [tb, p, c, t] with p the within-k-tile index
    xt8 = None
    xtb = None
    if n8:
        x8 = x.astype(NP_FP8)
        xt8 = np.ascontiguousarray(
            x8.reshape(t_tiles, P, k_tiles, P).transpose(0, 3, 2, 1)[:, :, :n8, :]
        )
    if nbf:
        xb = x.astype(NP_BF16)
        xtb = np.ascontiguousarray(
            xb.reshape(t_tiles, P, k_tiles, P).transpose(0, 3, 2, 1)[:, :, n8:, :]
        )
    in_maps = []
    for c in range(n_cores):
        w_c = weight[c * o_shard:(c + 1) * o_shard, :]  # [2048, 4096]
        # wt[p, kt, o] = W[o, kt*128 + p]
        wt_c = np.ascontiguousarray(
            w_c.T.reshape(k_tiles, P, o_shard).transpose(1, 0, 2)
        )
        wtb_c = wt_c.astype(NP_BF16)
        bias_c = np.ascontiguousarray(bias[c * o_shard:(c + 1) * o_shard]).reshape(1, o_shard)
        m = {"wt": wt_c, "wtb": wtb_c, "bias": bias_c}
        if n8:
            m["xt8"] = xt8
        if nbf:
            m["xtb"] = xtb
        in_maps.append(m)
    return in_maps


_CACHED_NC = None


def kernel(x: np.ndarray, weight: np.ndarray, bias: np.ndarray) -> np.ndarray:
    global _CACHED_NC
    if _CACHED_NC is None:
        _CACHED_NC = build_kernel()
    nc = _CACHED_NC
    in_maps = make_in_maps(x, weight, bias)
    res = bass_utils.run_bass_kernel_spmd(nc, in_maps, core_ids=list(range(N_CORES)))
    y = np.concatenate([res.results[c]["y"] for c in range(N_CORES)], axis=1)
    y = np.asarray(y, dtype=np.float32)
    assert y.shape == (TOKENS, OUT_FEAT) and y.dtype == np.float32
    return y
